# revision 1
# baseline (speedup 1.0000x reference)
"""Complex-valued relative-position attention (nn_CAttention) on 8 TRN2 cores.

Sharding: batch (4) x head-half (2) -> 8 cores. Each core computes its
batch's projections for its 4 heads, full attention for those heads, and a
row-split partial output projection. Host sums the two partial outputs per
batch and restacks.

Design highlights:
  - float32r matmuls everywhere on the f32 path (full PE rate, ~1.2e-4 rnd).
  - Complex arithmetic packed into the contraction dim: dots lhsT is
    A_h = scale*[Qr^T; -Qi^T], so dots_r = A^T @ [Kr;Ki] and
    dots_i = A^T @ [Ki;-Kr] are single 128-deep accumulation groups.
  - Relative-position term via the reversed-extended-table skew: qrev is a
    matmul against a host-reversed clamped table, round-tripped through a
    DRAM slot (pitch 1152) and read back with access pattern
    [[1151,128],[1,1024]] @ offset 127 == qrel_ext[i, i-j+1023], fully
    contiguous per row. Clamping is exact via edge-replicated table rows.
  - Skew tiles are injected into the dots PSUM group with fp16 identity
    matmuls; magnitude is one custom DVE op (r^2+i^2); sqrt is computed as
    exp(0.5*ln(x)) so the ACT engine uses a single table set (ln+exp) with
    zero table switches; exp's accum_out yields the softmax row-sum free.
  - attn (fp16) is PE-transposed; [Vr|Vi]-packed AV emits transposed head
    outputs feeding the row-split output projection directly.
  - The whole attention phase is software-pipelined (PREF iterations of
    skew-roundtrip prefetch) over a single 2KB-slot SBUF pool shared with
    the (short-lived) input/weight tiles.
"""
import functools
import numpy as np

import concourse.bass as bass
import concourse.bacc as bacc
import concourse.mybir as mybir
import concourse.tile as tile
from concourse.bass_utils import run_bass_kernel_spmd
from concourse.masks import make_identity

F32 = mybir.dt.float32
F32R = mybir.dt.float32r
F16 = mybir.dt.float16
AF = mybir.ActivationFunctionType

HEADS, DH, MAX_POS = 8, 64, 512
B, N, DIM = 4, 1024, 512
HPC = 4            # heads per core
KT = 4             # dim k-tiles (512/128)
NT = 8             # n tiles (1024/128)
WIN = 1152         # qrel window width (>= 1151)
SCALE = DH ** (-0.5)
PREF = 2           # skew round-trip prefetch distance (iterations)


def register_mag2():
    from concourse import dve_ops
    from concourse.dve_spec import Spec, Src0, Src1, AluOp, Bin, lower, sq
    from concourse.dve_uop import DveOpSpec

    for op in dve_ops.OPS:
        if op.name == "MAG2_ANT":
            return op
    spec = Spec(
        body=Bin(AluOp.ADD, sq(Src0), sq(Src1)),
        reference=lambda in0, in1, s0, s1, imm2: (
            in0.astype(np.float32) ** 2 + in1.astype(np.float32) ** 2
        ),
    )
    opcode = dve_ops._CUSTOM_DVE_ROW_BASE + len(dve_ops.OPS)
    shas = {}
    for ver in ("v3",):
        s = DveOpSpec(name="MAG2_ANT", opcode=opcode,
                      uops=lower(spec, ver=ver), rd1_en=True)
        shas[ver] = s.sha(ver)
    op = dve_ops.DveOp("MAG2_ANT", spec, subdim=False, uops_sha=shas)
    dve_ops._SUB_OPCODE_FOR_NAME[op.name] = opcode
    dve_ops.OPS.append(op)
    dve_ops.CUSTOM_DVE_SPECS[op.name] = op.spec

    spec2 = Spec(
        body=sq(Bin(AluOp.ADD, Src0, Src1)),
        reference=lambda in0, in1, s0, s1, imm2: (
            (in0.astype(np.float32) + in1.astype(np.float32)) ** 2
        ),
    )
    opcode2 = dve_ops._CUSTOM_DVE_ROW_BASE + len(dve_ops.OPS)
    shas2 = {}
    for ver in ("v3",):
        s2 = DveOpSpec(name="ADDSQ_ANT", opcode=opcode2,
                       uops=lower(spec2, ver=ver), rd1_en=True)
        shas2[ver] = s2.sha(ver)
    op2 = dve_ops.DveOp("ADDSQ_ANT", spec2, subdim=False, uops_sha=shas2)
    dve_ops._SUB_OPCODE_FOR_NAME[op2.name] = opcode2
    dve_ops.OPS.append(op2)
    dve_ops.CUSTOM_DVE_SPECS[op2.name] = op2.spec

    spec3 = Spec(
        body=Bin(AluOp.ADD, Src0, sq(Src1)),
        reference=lambda in0, in1, s0, s1, imm2: (
            in0.astype(np.float32) + in1.astype(np.float32) ** 2
        ),
    )
    opcode3 = dve_ops._CUSTOM_DVE_ROW_BASE + len(dve_ops.OPS)
    shas3 = {}
    for ver in ("v3",):
        s3 = DveOpSpec(name="SQACC_ANT", opcode=opcode3,
                       uops=lower(spec3, ver=ver), rd1_en=True)
        shas3[ver] = s3.sha(ver)
    op3 = dve_ops.DveOp("SQACC_ANT", spec3, subdim=False, uops_sha=shas3)
    dve_ops._SUB_OPCODE_FOR_NAME[op3.name] = opcode3
    dve_ops.OPS.append(op3)
    dve_ops.CUSTOM_DVE_SPECS[op3.name] = op3.spec
    return op, op2, op3


def c_lo(i_blk):
    return 896 - 128 * i_blk


@functools.cache
def build_module():
    import concourse.tile_utils as tile_utils
    if getattr(tile_utils, "max_sbuf_usage", 0) < 208 * 1024:
        tile_utils.max_sbuf_usage = 208 * 1024

    # Pin the ACT engine to the ln+exp table set: every activation this
    # kernel emits (Ln, Exp, Copy/Identity) lives in that one set, so the
    # greedy table-load pass emits exactly one LoadActFuncSet.
    import concourse.bacc as bacc_mod
    if not getattr(bacc_mod, "_ant_act_tables_pinned", False):
        orig_gat = bacc_mod.get_activation_tables

        def pinned_gat(arch):
            # Keep the full set list (ids must match act_info.json order);
            # strip Ln/Exp from every other set so the load-insertion pass
            # can only satisfy them with the dual ln+exp set.
            full = orig_gat(arch)
            out = {}
            for name, funcs in full.items():
                if name != "natural_log_exp_and_others":
                    funcs = funcs - {mybir.ActivationFunctionType.Ln,
                                     mybir.ActivationFunctionType.Exp}
                out[name] = funcs
            return out

        bacc_mod.get_activation_tables = pinned_gat
        bacc_mod._ant_act_tables_pinned = True

    mag2, addsq, sqacc = register_mag2()
    nc = bacc.Bacc("TRN2", target_bir_lowering=False, debug=False,
                   num_devices=8)

    din = {}
    for nm, shape, dt_ in [
        ("xt_r", [DIM, N], F32R), ("xt_i", [DIM, N], F32R),
        ("wq_a", [DIM, 512], F32R), ("wq_b", [DIM, 512], F32R),
        ("wk_a", [DIM, 512], F32R), ("wk_b", [DIM, 512], F32R),
        ("wv_a", [DIM, 512], F32R), ("wv_b", [DIM, 512], F32R),
        ("wo_s", [DIM, 512], F32R),
        ("rel_r", [128, 2048], F32R), ("rel_i", [128, 2048], F32R),
        ("bo_rt", [128, 4], F32), ("bo_it", [128, 4], F32),
        ("smask", [128, 1], F32),
    ]:
        din[nm] = nc.dram_tensor(nm, shape, dt_, kind="ExternalInput")
    o_r = nc.dram_tensor("o_r", [DIM, N], F32, kind="ExternalOutput")
    o_i = nc.dram_tensor("o_i", [DIM, N], F32, kind="ExternalOutput")

    with tile.TileContext(nc) as tc:
        with (
            tc.tile_pool(name="const", bufs=1) as cpool,
            tc.tile_pool(name="work", bufs=44) as pw,     # 2KB slots
            tc.tile_pool(name="qev", bufs=4) as pqe,      # 2.25KB slots
            tc.tile_pool(name="stacks", bufs=14) as pstk,  # 4KB slots
            tc.tile_pool(name="otp", bufs=6) as pot,      # 4KB slots
            tc.tile_pool(name="outsb", bufs=2) as pout,
            tc.tile_pool(name="small", bufs=12) as psm,
            tc.tile_pool(name="psB", bufs=2, space="PSUM") as psB,
            tc.tile_pool(name="psC", bufs=2, space="PSUM") as psC,
            tc.tile_pool(name="psQ", bufs=1, space="PSUM") as psQ,
            tc.tile_pool(name="dram", bufs=10, space="DRAM") as pdram,
        ):
            # ---------------- constants ----------------
            id16 = cpool.tile([128, 128], F16, tag="id16")
            make_identity(nc, id16[:])
            rel_r = cpool.tile([128, 2048], F32R, tag="rel_r")
            rel_i = cpool.tile([128, 2048], F32R, tag="rel_i")
            nc.sync.dma_start(rel_r[:], din["rel_r"][:, :])
            nc.scalar.dma_start(rel_i[:], din["rel_i"][:, :])
            bo_rt = cpool.tile([128, 4], F32, tag="bo_rt")
            bo_it = cpool.tile([128, 4], F32, tag="bo_it")
            smask = cpool.tile([128, 1], F32, tag="smask")
            nc.sync.dma_start(bo_rt[:], din["bo_rt"][:, :])
            nc.sync.dma_start(bo_it[:], din["bo_it"][:, :])
            nc.sync.dma_start(smask[:], din["smask"][:, :])
            wo_s = cpool.tile([128, 4, 512], F32R, tag="wo_s")
            for kt in range(KT):
                nc.scalar.dma_start(wo_s[:, kt, :],
                                    din["wo_s"][kt * 128:(kt + 1) * 128, :])

            engs = (nc.sync, nc.scalar, nc.gpsimd)

            # xt tiles: [128,512] per (r/i, kt, nh)
            xt = {}
            qd = 0
            for nm in ("xt_r", "xt_i"):
                for kt in range(KT):
                    for nh in range(2):
                        t = pw.tile([128, 512], F32R, tag="wk",
                                    name=f"{nm}_{kt}_{nh}")
                        engs[qd % 3].dma_start(
                            t[:], bass.AP(din[nm], kt * 128 * N + nh * 512,
                                          [[N, 128], [1, 512]]))
                        qd += 1
                        xt[(nm, kt, nh)] = t

            def load_w(nm):
                nonlocal qd
                out = []
                for kt in range(KT):
                    t = pw.tile([128, 512], F32R, tag="wk",
                                name=f"{nm}_{kt}")
                    engs[qd % 3].dma_start(
                        t[:], din[nm][kt * 128:(kt + 1) * 128, :])
                    qd += 1
                    out.append(t)
                return out

            # ---------------- phase P: projections ----------------
            A = [None] * HPC
            Knat = [None] * HPC
            Kni2 = [None] * HPC
            Vp = [None] * HPC

            for kind in ("q", "k"):
                wa = load_w(f"w{kind}_a")
                wb = load_w(f"w{kind}_b")
                for h in range(HPC):
                    hs = slice(h * 128, (h + 1) * 128)
                    if kind == "q":
                        A[h] = pstk.tile([128, 1024], F32R, tag="stk",
                                         name=f"A{h}")
                    else:
                        Knat[h] = pstk.tile([128, 1024], F32R, tag="stk",
                                            name=f"Knat{h}")
                        Kni2[h] = pstk.tile([128, 1024], F32R, tag="stk",
                                            name=f"Kni2{h}")
                    for nh in range(2):
                        ns = slice(nh * 512, (nh + 1) * 512)
                        ps = psB.tile([128, 512], F32, tag="pb",
                                      name=f"ps{kind}_{h}_{nh}")
                        for kt in range(KT):
                            nc.tensor.matmul(ps[:], wa[kt][:, hs],
                                             xt[("xt_r", kt, nh)][:],
                                             start=(kt == 0), stop=False)
                        for kt in range(KT):
                            nc.tensor.matmul(ps[:], wb[kt][:, hs],
                                             xt[("xt_i", kt, nh)][:],
                                             start=False, stop=(kt == KT - 1))
                        if kind == "q":
                            nc.vector.tensor_scalar_mul(A[h][:, ns], ps[:], smask[:])
                        else:
                            nc.scalar.copy(Knat[h][:, ns], ps[:])
                            nc.scalar.copy(Kni2[h][0:64, ns], ps[64:128, :])
                            nc.scalar.mul(Kni2[h][64:128, ns],
                                          ps[0:64, :], -1.0)

            wva = load_w("wv_a")
            wvb = load_w("wv_b")
            Vpp = [pstk.tile([128, 8, 256], F16, tag="stk", name=f"Vpp{p}")
                   for p in range(2)]

            def emit_vproj(p, J):
                # two heads per matmul so N=256 keeps float32r at full rate
                hs = slice(p * 256, (p + 1) * 256)
                xs = slice((J % 4) * 128, (J % 4) * 128 + 128)
                vps = psB.tile([128, 256], F32, tag="pb",
                               name=f"vps_{p}_{J}")
                for kt in range(KT):
                    nc.tensor.matmul(vps[:],
                                     xt[("xt_r", kt, J // 4)][:, xs],
                                     wva[kt][:, hs],
                                     start=(kt == 0), stop=False)
                for kt in range(KT):
                    nc.tensor.matmul(vps[:],
                                     xt[("xt_i", kt, J // 4)][:, xs],
                                     wvb[kt][:, hs],
                                     start=False, stop=(kt == KT - 1))
                nc.vector.tensor_copy(Vpp[p][:, J, :], vps[:])

            vunits = [(p, J) for p in range(2) for J in range(NT)]

            # OT stacks for the output projection
            OT_A = [pot.tile([128, 1024], F32R, tag="ot4", name=f"OTA{t}")
                    for t in range(4)]
            OT_B = [pot.tile([128, 1024], F32R, tag="ot4", name=f"OTB{t}")
                    for t in range(2)]

            # ---------------- phase A: pipelined attention ----------------
            def emit_qrel(h, I):
                isl = slice(I * 128, (I + 1) * 128)
                lo = c_lo(I)
                out = {}
                for part, relt in (("r", rel_r), ("i", rel_i)):
                    slot = pdram.tile([128, WIN], F16, tag="qrev",
                                      name=f"qrev{part}_{h}_{I}")
                    qe = pqe.tile([128, WIN], F16, tag="qe",
                                  name=f"qe{part}_{h}_{I}")
                    for ch in range(2):
                        qps = psQ.tile([128, 576], F32, tag="pq",
                                       name=f"qps{part}_{h}_{I}_{ch}")
                        base = ch * 576
                        nc.tensor.matmul(
                            qps[:, 0:512], A[h][:, isl],
                            relt[:, lo + base:lo + base + 512],
                            start=True, stop=True)
                        nc.tensor.matmul(
                            qps[:, 512:576], A[h][:, isl],
                            relt[:, lo + base + 512:lo + base + 576],
                            start=True, stop=True)
                        if part == "r":
                            nc.vector.tensor_copy(qe[:, base:base + 576],
                                                  qps[:])
                        else:
                            nc.scalar.copy(qe[:, base:base + 576], qps[:])
                    nc.gpsimd.dma_start(slot[:, :], qe[:])
                    skw = pw.tile([128, 1024], F16, tag="wk",
                                  name=f"skew{part}_{h}_{I}")
                    nc.sync.dma_start(
                        skw[:],
                        bass.AP(slot.tensor, 127, [[WIN - 1, 128], [1, 1024]]))
                    out[part] = skw
                return out

            def stage_B(h, I, skw):
                isl = slice(I * 128, (I + 1) * 128)
                er = pw.tile([128, 1024], F16, tag="wk", name=f"er_{h}_{I}")
                dpsr = psB.tile([128, 1024], F32, tag="pb",
                                name=f"dpsr_{h}_{I}")
                for nh in range(2):
                    ns = slice(nh * 512, (nh + 1) * 512)
                    nc.tensor.matmul(dpsr[:, ns], A[h][:, isl],
                                     Knat[h][:, ns], start=True, stop=True)
                nc.vector._custom_dve(addsq, out=er[:],
                                      in0=skw["r"][:], in1=dpsr[:])
                dpsi = psB.tile([128, 1024], F32, tag="pb",
                                name=f"dpsi_{h}_{I}")
                for nh in range(2):
                    ns = slice(nh * 512, (nh + 1) * 512)
                    nc.tensor.matmul(dpsi[:, ns], A[h][:, isl],
                                     Kni2[h][:, ns], start=True, stop=True)
                ei = pw.tile([128, 1024], F16, tag="wk", name=f"ei_{h}_{I}")
                nc.vector._custom_dve(addsq, out=ei[:],
                                      in0=skw["i"][:], in1=dpsi[:])
                m2 = pw.tile([128, 1024], F16, tag="wk", name=f"m2_{h}_{I}")
                nc.vector.tensor_add(m2[:], er[:], ei[:])
                # sqrt via exp(0.5*ln): single ACT table set for whole kernel
                lt = pw.tile([128, 1024], F16, tag="wk", name=f"lt_{h}_{I}")
                nc.scalar.activation(lt[:], m2[:], AF.Ln)
                mt = pw.tile([128, 1024], F16, tag="wk", name=f"mt_{h}_{I}")
                nc.scalar.activation(mt[:], lt[:], AF.Exp, scale=0.5)
                attn = pw.tile([128, 1024], F16, tag="wk",
                               name=f"attn_{h}_{I}")
                rs = psm.tile([128, 1], F32, tag="sm", name=f"rs_{h}_{I}")
                nc.scalar.activation(attn[:], mt[:], AF.Exp, accum_out=rs[:])
                rc = psm.tile([128, 1], F32, tag="sm", name=f"rc_{h}_{I}")
                nc.vector.reciprocal(rc[:], rs[:])
                return {"attn": attn, "rc": rc}

            def stage_C(h, I, st):
                attn, rc = st["attn"], st["rc"]
                nc.vector.tensor_scalar_mul(attn[:], attn[:], rc[:])
                tps = psC.tile([128, 1024], F16, tag="pc", name=f"tps_{h}_{I}")
                for J in range(NT):
                    js = slice(J * 128, (J + 1) * 128)
                    nc.tensor.transpose(tps[:, js], attn[:, js], id16[:])
                atT = pw.tile([128, 1024], F16, tag="wk", name=f"atT_{h}_{I}")
                nc.vector.tensor_copy(atT[:], tps[:])
                return atT

            def stage_D(h, I, atT):
                isl = slice(I * 128, (I + 1) * 128)
                avs = psC.tile([128, 128], F32, tag="pc", name=f"avs_{h}_{I}")
                vsl = slice((h % 2) * 128, (h % 2) * 128 + 128)
                for J in range(NT):
                    js = slice(J * 128, (J + 1) * 128)
                    nc.tensor.matmul(avs[:], Vpp[h // 2][:, J, vsl],
                                     atT[:, js],
                                     start=(J == 0), stop=(J == NT - 1))
                prt = slice((h % 2) * 64, (h % 2) * 64 + 64)
                nc.vector.tensor_copy(OT_A[h // 2][prt, isl], avs[0:64, :])
                nc.vector.tensor_scalar_mul(OT_A[2 + h // 2][prt, isl],
                                            avs[64:128, :], -1.0)
                nc.vector.tensor_copy(OT_B[h // 2][prt, isl], avs[64:128, :])

            def emit_outproj(nh):
                ns = slice(nh * 512, (nh + 1) * 512)
                for part, bo_t in (("r", bo_rt), ("i", bo_it)):
                    for dt_ in range(4):
                        ds = slice(dt_ * 128, (dt_ + 1) * 128)
                        ops = psC.tile([128, 512], F32, tag="pc",
                                       name=f"ops_{part}_{dt_}_{nh}")
                        if part == "r":
                            rhs = [OT_A[0], OT_A[1], OT_A[2], OT_A[3]]
                        else:
                            rhs = [OT_B[0], OT_B[1], OT_A[0], OT_A[1]]
                        for j, rtt in enumerate(rhs):
                            nc.tensor.matmul(ops[:], wo_s[:, j, ds],
                                             rtt[:, ns],
                                             start=(j == 0), stop=(j == 3))
                        osb = pout.tile([128, 512], F32, tag="ot",
                                        name=f"osb_{part}_{dt_}_{nh}")
                        nc.vector.tensor_scalar_add(osb[:], ops[:],
                                                    bo_t[:, dt_:dt_ + 1])
                        dst = o_r if part == "r" else o_i
                        nc.sync.dma_start(
                            bass.AP(dst, dt_ * 128 * N + nh * 512,
                                    [[N, 128], [1, 512]]),
                            osb[:])

            flat = [(h, I) for h in range(HPC) for I in range(NT)]
            PB, PC, PD = PREF, PREF + 2, PREF + 3
            skewmap, bmap, cmap = {}, {}, {}
            for s in range(len(flat) + PD + 1):
                for _ in range(2):
                    if vunits:
                        emit_vproj(*vunits.pop(0))
                if s < len(flat):
                    h, I = flat[s]
                    skewmap[(h, I)] = emit_qrel(h, I)
                if PB <= s < len(flat) + PB:
                    h, I = flat[s - PB]
                    bmap[(h, I)] = stage_B(h, I, skewmap.pop((h, I)))
                if PC <= s < len(flat) + PC:
                    h, I = flat[s - PC]
                    cmap[(h, I)] = stage_C(h, I, bmap.pop((h, I)))
                if PD <= s < len(flat) + PD:
                    h, I = flat[s - PD]
                    stage_D(h, I, cmap.pop((h, I)))
                    if (h, I) == (HPC - 1, 3):
                        emit_outproj(0)
            emit_outproj(1)

    nc.compile()
    return nc, mag2


def _prep_core_inputs(inputs, core):
    b, half = core // 2, core % 2
    x = inputs["x"]
    f32 = np.float32
    xt_r = np.ascontiguousarray(x[b, :, :, 0].T).astype(f32)
    xt_i = np.ascontiguousarray(x[b, :, :, 1].T).astype(f32)

    def pack_ab(wr, wi):
        a = np.empty((DIM, 512), f32)
        bb = np.empty((DIM, 512), f32)
        for hl in range(HPC):
            gh = half * HPC + hl
            cs = slice(gh * DH, (gh + 1) * DH)
            a[:, hl * 128:hl * 128 + 64] = wr[:, cs]
            a[:, hl * 128 + 64:hl * 128 + 128] = wi[:, cs]
            bb[:, hl * 128:hl * 128 + 64] = -wi[:, cs]
            bb[:, hl * 128 + 64:hl * 128 + 128] = wr[:, cs]
        return a, bb

    wq_a, wq_b = pack_ab(inputs["wq_r"], inputs["wq_i"])
    wk_a, wk_b = pack_ab(inputs["wkv_r"][:, :512], inputs["wkv_i"][:, :512])
    wv_a, wv_b = pack_ab(inputs["wkv_r"][:, 512:], inputs["wkv_i"][:, 512:])

    rs = slice(half * 256, (half + 1) * 256)
    wo_s = np.concatenate(
        [inputs["wo_r"][rs, :], inputs["wo_i"][rs, :]], 0).astype(f32)

    e = np.arange(2047)
    t_ext = inputs["rel_emb"][np.clip(e - 1023, -MAX_POS, MAX_POS) + MAX_POS]
    relrev = t_ext[::-1].astype(f32)           # [2047, 64]
    rel_r = np.zeros((128, 2048), f32)
    rel_i = np.zeros((128, 2048), f32)
    rel_r[0:64, 0:2047] = relrev.T
    rel_i[64:128, 0:2047] = -relrev.T

    bscale = 1.0 if half == 0 else 0.0
    bo_rt = np.ascontiguousarray(
        inputs["bo_r"].reshape(4, 128).T * bscale).astype(f32)
    bo_it = np.ascontiguousarray(
        inputs["bo_i"].reshape(4, 128).T * bscale).astype(f32)
    smask = np.concatenate(
        [np.full(64, SCALE, f32), np.full(64, -SCALE, f32)]).reshape(128, 1)

    return {
        "xt_r": xt_r, "xt_i": xt_i,
        "wq_a": wq_a, "wq_b": wq_b, "wk_a": wk_a, "wk_b": wk_b,
        "wv_a": wv_a, "wv_b": wv_b, "wo_s": wo_s,
        "rel_r": rel_r, "rel_i": rel_i,
        "bo_rt": bo_rt, "bo_it": bo_it, "smask": smask,
    }


_last_results = {}


def kernel(**inputs):
    inputs = {k: np.asarray(v) for k, v in inputs.items()}
    nc, _ = build_module()
    in_maps = [_prep_core_inputs(inputs, c) for c in range(8)]
    res = run_bass_kernel_spmd(nc, in_maps, core_ids=list(range(8)))
    _last_results["res"] = res

    out = np.empty((B, N, DIM, 2), np.float32)
    for b in range(B):
        r = res.results[2 * b]["o_r"] + res.results[2 * b + 1]["o_r"]
        i = res.results[2 * b]["o_i"] + res.results[2 * b + 1]["o_i"]
        out[b, :, :, 0] = r.T
        out[b, :, :, 1] = i.T
    return out



# revision 29
# speedup vs baseline: 1.2817x; 1.2817x over previous
"""Complex-valued relative-position attention (nn_CAttention) on 8 TRN2 cores.

Sharding: batch (4) x head-half (2) -> 8 cores. Each core computes its
batch's projections for its 4 heads, full attention for those heads, and a
row-split partial output projection. Host sums the two partial outputs per
batch and restacks.

v3 design:
  - fp16 datapath end to end (x, projection weights, A/K stacks, rel
    tables): every matmul runs 1 cycle/row at any moving size.
  - Skew (rel-pos) terms are DMA-round-tripped through DRAM (diagonal AP
    read) then INJECTED into the dots PSUM group with identity matmuls, so
    the magnitude becomes two half-width custom DVE mag2 ops (dr^2+di^2)
    reading PSUM directly -- no adds on DVE.
  - qrel windows r+i share one SBUF qe tile per unit; PAIRS of units share
    one DRAM slot: one slot write + one diagonal read per two units.
  - Q/K projections for heads 1-3 are interleaved into the pipeline, so the
    startup is just x/weight loads + head-0 projections.
  - Output projection uses a negated-wo_i weight copy instead of negated AV
    stacks; osb bias-adds on ACT (idle at the tail).
  - Unified 8x1-bank PSUM pool for maximum slot fungibility.
"""
import functools
import numpy as np

import concourse.bass as bass
import concourse.bacc as bacc
import concourse.mybir as mybir
import concourse.tile as tile
from concourse.bass_utils import run_bass_kernel_spmd
from concourse.masks import make_identity

F32 = mybir.dt.float32
F16 = mybir.dt.float16
AF = mybir.ActivationFunctionType

HEADS, DH, MAX_POS = 8, 64, 512
B, N, DIM = 4, 1024, 512
HPC = 4            # heads per core
KT = 4             # dim k-tiles (512/128)
NT = 8             # n tiles (1024/128)
SCALE = DH ** (-0.5)


def register_custom_ops():
    from concourse import dve_ops
    from concourse.dve_spec import Spec, Src0, Src1, AluOp, Bin, lower, sq
    from concourse.dve_uop import DveOpSpec

    def reg(name, body, ref):
        for op in dve_ops.OPS:
            if op.name == name:
                return op
        spec = Spec(body=body, reference=ref)
        opcode = dve_ops._CUSTOM_DVE_ROW_BASE + len(dve_ops.OPS)
        shas = {}
        for ver in ("v3",):
            s = DveOpSpec(name=name, opcode=opcode,
                          uops=lower(spec, ver=ver), rd1_en=True)
            shas[ver] = s.sha(ver)
        op = dve_ops.DveOp(name, spec, subdim=False, uops_sha=shas)
        dve_ops._SUB_OPCODE_FOR_NAME[op.name] = opcode
        dve_ops.OPS.append(op)
        dve_ops.CUSTOM_DVE_SPECS[op.name] = op.spec
        return op

    mag2 = reg(
        "MAG2_ANT",
        Bin(AluOp.ADD, sq(Src0), sq(Src1)),
        lambda in0, in1, s0, s1, imm2: (
            in0.astype(np.float32) ** 2 + in1.astype(np.float32) ** 2),
    )
    return mag2


def c_lo(i_blk):
    return 896 - 128 * i_blk


@functools.cache
def build_module():
    import concourse.tile_utils as tile_utils
    if getattr(tile_utils, "max_sbuf_usage", 0) < 208 * 1024:
        tile_utils.max_sbuf_usage = 208 * 1024

    # Pin the ACT engine to the ln+exp table set: every activation this
    # kernel emits (Ln, Exp, Copy/Identity) lives in that one set, so the
    # greedy table-load pass emits exactly one LoadActFuncSet.
    import concourse.bacc as bacc_mod
    if not getattr(bacc_mod, "_ant_act_tables_pinned", False):
        orig_gat = bacc_mod.get_activation_tables

        def pinned_gat(arch):
            full = orig_gat(arch)
            out = {}
            for name, funcs in full.items():
                if name != "natural_log_exp_and_others":
                    funcs = funcs - {mybir.ActivationFunctionType.Ln,
                                     mybir.ActivationFunctionType.Exp}
                out[name] = funcs
            return out

        bacc_mod.get_activation_tables = pinned_gat
        bacc_mod._ant_act_tables_pinned = True

    mag2 = register_custom_ops()
    nc = bacc.Bacc("TRN2", target_bir_lowering=False, debug=False,
                   num_devices=8)

    din = {}
    for nm, shape, dt_ in [
        ("xt_r", [DIM, N], F16), ("xt_i", [DIM, N], F16),
        ("wq_a", [DIM, 512], F16), ("wq_b", [DIM, 512], F16),
        ("wk_a", [DIM, 512], F16), ("wk_b", [DIM, 512], F16),
        ("wv_a", [DIM, 512], F16), ("wv_b", [DIM, 512], F16),
        ("wo_st", [DIM, 512], F16), ("wo_sn", [DIM, 512], F16),
        ("rel_r", [128, 2048], F16), ("rel_i", [128, 2048], F16),
        ("bo_rt", [128, 4], F32), ("bo_it", [128, 4], F32),
        ("smask", [128, 1], F32),
    ]:
        din[nm] = nc.dram_tensor(nm, shape, dt_, kind="ExternalInput")
    o_r = nc.dram_tensor("o_r", [DIM, N], F32, kind="ExternalOutput")
    o_i = nc.dram_tensor("o_i", [DIM, N], F32, kind="ExternalOutput")

    with tile.TileContext(nc) as tc:
        with (
            tc.tile_pool(name="const", bufs=1) as cpool,
            tc.tile_pool(name="xw", bufs=8) as pxw,
            tc.tile_pool(name="ww", bufs=6) as pww,      # 1KB fp16 x/w
            tc.tile_pool(name="work", bufs=26) as pw,     # 2KB slots
            tc.tile_pool(name="qev", bufs=3) as pqe,      # 9KB pair slots
            tc.tile_pool(name="skw", bufs=3) as pskw,     # 8KB pair slots
            tc.tile_pool(name="stacks", bufs=12) as pstk,  # 2KB fp16 stacks
            tc.tile_pool(name="vpp", bufs=2) as pvp,      # 4KB
            tc.tile_pool(name="otp", bufs=4) as pot,      # 2KB fp16
            tc.tile_pool(name="outsb", bufs=4) as pout,
            tc.tile_pool(name="small", bufs=12) as psm,
            tc.tile_pool(name="psU", bufs=4, space="PSUM") as psU,
            tc.tile_pool(name="psD", bufs=4, space="PSUM") as psD,
            tc.tile_pool(name="dram", bufs=4, space="DRAM") as pdram,
        ):
            # ---------------- constants ----------------
            id16 = cpool.tile([128, 128], F16, tag="id16")
            make_identity(nc, id16[:])
            smask = cpool.tile([128, 1], F32, tag="smask")
            nc.sync.dma_start(smask[:], din["smask"][:, :])
            engs = (nc.sync, nc.scalar, nc.gpsimd)

            # x tiles: [128,1024] fp16 per (r/i, kt); weights [128,4,512]
            xt = {}
            qd = 0

            def load_w(nm):
                nonlocal qd
                t = pww.tile([128, 4, 512], F16, tag="w4", name=f"w_{nm}")
                engs[qd % 3].dma_start(
                    t[:, :, :],
                    bass.AP(din[nm], 0, [[512, 128], [512 * 128, 4], [1, 512]]))
                qd += 1
                return t

            def load_xt():
                nonlocal qd
                for nm in ("xt_r", "xt_i"):
                    for kt in range(KT):
                        t = pxw.tile([128, 1024], F16, tag="xw",
                                     name=f"{nm}_{kt}")
                        engs[qd % 3].dma_start(
                            t[:], bass.AP(din[nm], kt * 128 * N,
                                          [[N, 128], [1, 1024]]))
                        qd += 1
                        xt[(nm, kt)] = t

            wq = (load_w("wq_a"), load_w("wq_b"))
            load_xt()
            wk = (load_w("wk_a"), load_w("wk_b"))
            wv = (load_w("wv_a"), load_w("wv_b"))

            rel_r = cpool.tile([128, 2048], F16, tag="rel_r")
            rel_i = cpool.tile([128, 2048], F16, tag="rel_i")
            nc.sync.dma_start(rel_r[:], din["rel_r"][:, :])
            nc.scalar.dma_start(rel_i[:], din["rel_i"][:, :])
            bo_rt = cpool.tile([128, 4], F32, tag="bo_rt")
            bo_it = cpool.tile([128, 4], F32, tag="bo_it")
            nc.sync.dma_start(bo_rt[:], din["bo_rt"][:, :])
            nc.sync.dma_start(bo_it[:], din["bo_it"][:, :])
            wo_st = cpool.tile([128, 4, 512], F16, tag="wo_st")
            wo_sn = cpool.tile([128, 4, 512], F16, tag="wo_sn")
            nc.scalar.dma_start(
                wo_st[:, :, :],
                bass.AP(din["wo_st"], 0, [[512, 128], [512 * 128, 4], [1, 512]]))
            nc.scalar.dma_start(
                wo_sn[:, :, :],
                bass.AP(din["wo_sn"], 0, [[512, 128], [512 * 128, 4], [1, 512]]))

            # ---------------- projections (emitted per head) ----------------
            A16 = [None] * HPC
            Kn16 = [None] * HPC
            Ki16 = [None] * HPC

            def emit_proj(kind, h):
                wa, wb = wq if kind == "q" else wk
                hs = slice(h * 128, (h + 1) * 128)
                if kind == "q":
                    A16[h] = pstk.tile([128, 1024], F16, tag="stk",
                                       name=f"A{h}")
                else:
                    Kn16[h] = pstk.tile([128, 1024], F16, tag="stk",
                                        name=f"Kn{h}")
                    Ki16[h] = pstk.tile([128, 1024], F16, tag="stk",
                                        name=f"Ki{h}")
                for nh in range(2):
                    ns = slice(nh * 512, (nh + 1) * 512)
                    ps = psU.tile([128, 512], F32, tag="pu",
                                  name=f"ps{kind}_{h}_{nh}")
                    for kt in range(KT):
                        nc.tensor.matmul(ps[:], wa[:, kt, hs],
                                         xt[("xt_r", kt)][:, ns],
                                         start=(kt == 0), stop=False)
                    for kt in range(KT):
                        nc.tensor.matmul(ps[:], wb[:, kt, hs],
                                         xt[("xt_i", kt)][:, ns],
                                         start=False, stop=(kt == KT - 1))
                    if kind == "q":
                        nc.scalar.mul(A16[h][:, ns], ps[:], smask[:, 0:1])
                    else:
                        nc.scalar.copy(Kn16[h][:, ns], ps[:])
                        nc.scalar.copy(Ki16[h][0:64, ns], ps[64:128, :])
                        nc.scalar.mul(Ki16[h][64:128, ns],
                                      ps[0:64, :], -1.0)

            Vpp = [pvp.tile([128, 8, 256], F16, tag="vpp", name=f"Vpp{p}")
                   for p in range(2)]

            def emit_vproj(p, J):
                hs = slice(p * 256, (p + 1) * 256)
                xs = slice((J % 4) * 128 + (J // 4) * 512,
                           (J % 4) * 128 + (J // 4) * 512 + 128)
                vps = psU.tile([128, 256], F32, tag="pu",
                               name=f"vps_{p}_{J}")
                for kt in range(KT):
                    nc.tensor.matmul(vps[:], xt[("xt_r", kt)][:, xs],
                                     wv[0][:, kt, hs],
                                     start=(kt == 0), stop=False)
                for kt in range(KT):
                    nc.tensor.matmul(vps[:], xt[("xt_i", kt)][:, xs],
                                     wv[1][:, kt, hs],
                                     start=False, stop=(kt == KT - 1))
                nc.vector.tensor_copy(Vpp[p][:, J, :], vps[:])

            # OT stacks for the output projection (r and i AV halves)
            OT_A = [pot.tile([128, 1024], F16, tag="ot", name=f"OTA{t}")
                    for t in range(2)]
            OT_B = [pot.tile([128, 1024], F16, tag="ot", name=f"OTB{t}")
                    for t in range(2)]

            # ---------------- attention pipeline stages ----------------
            # Pairs of units share one qe tile / DRAM slot / skw tile:
            # layout per row: [u0-r 1152 | u0-i 1152 | u1-r 1152 | u1-i 1152]
            def emit_qrel_qe(h, I, qe, half):
                isl = slice(I * 128, (I + 1) * 128)
                lo = c_lo(I)
                for part, relt in ((0, rel_r), (1, rel_i)):
                    for c0, c1 in ((0, 512), (512, 1024), (1024, 1152)):
                        qps = psU.tile([128, c1 - c0], F32, tag="pu",
                                       name=f"qps{part}_{c0}_{h}_{I}")
                        nc.tensor.matmul(qps[:], A16[h][:, isl],
                                         relt[:, lo + c0:lo + c1],
                                         start=True, stop=True)
                        qs = slice(2304 * half + 1152 * part + c0,
                                   2304 * half + 1152 * part + c1)
                        if part == 0:
                            nc.vector.tensor_copy(qe[:, qs], qps[:])
                        else:
                            nc.scalar.copy(qe[:, qs], qps[:])

            def emit_skwread(slot):
                skw = pskw.tile([128, 4096], F16, tag="skw",
                                name=f"skw_{slot.name}")
                nc.sync.dma_start(
                    skw[:],
                    bass.AP(slot.tensor, 127,
                            [[4607, 128], [1152, 4], [1, 1024]]))
                return skw

            def emit_dots(h, I, skw, half):
                isl = slice(I * 128, (I + 1) * 128)
                out = []
                for part, K in ((0, Kn16[h]), (1, Ki16[h])):
                    for nh in range(2):
                        ns = slice(nh * 512, (nh + 1) * 512)
                        d = psD.tile([128, 512], F32, tag="pd",
                                     name=f"dps{part}_{nh}_{h}_{I}")
                        ss = 2048 * half + 1024 * part + 512 * nh
                        if part == 0:
                            # r-part: skew injected into PSUM by identity mm
                            nc.tensor.matmul(d[:], A16[h][:, isl], K[:, ns],
                                             start=True, stop=False)
                            nc.tensor.matmul(d[:], id16[:],
                                             skw[:, ss:ss + 512],
                                             start=False, stop=True)
                        else:
                            nc.tensor.matmul(d[:], A16[h][:, isl], K[:, ns],
                                             start=True, stop=True)
                        out.append(d)
                # i-part: skew add fused with the PSUM->SBUF staging (custom
                # DVE ops allow at most one PSUM operand, so MAG2 needs the
                # i-dots in SBUF anyway).
                ei = pw.tile([128, 1024], F16, tag="wk", name=f"ei_{h}_{I}")
                ss = 2048 * half + 1024
                nc.vector.tensor_add(ei[:, 0:512], out[2][:],
                                     skw[:, ss:ss + 512])
                nc.vector.tensor_add(ei[:, 512:1024], out[3][:],
                                     skw[:, ss + 512:ss + 1024])
                return out, ei

            def emit_mag(h, I, dps_ei):
                dps, ei = dps_ei
                m2 = pw.tile([128, 1024], F16, tag="wk", name=f"m2_{h}_{I}")
                for nh in range(2):
                    ns = slice(nh * 512, (nh + 1) * 512)
                    nc.vector._custom_dve(mag2, out=m2[:, ns],
                                          in0=ei[:, ns], in1=dps[nh][:])
                return m2

            def emit_soft(h, I, m2):
                # sqrt via exp(0.5*ln): single ACT table set for whole kernel
                lt = pw.tile([128, 1024], F16, tag="wk", name=f"lt_{h}_{I}")
                nc.scalar.activation(lt[:], m2[:], AF.Ln)
                mt = pw.tile([128, 1024], F16, tag="wk", name=f"mt_{h}_{I}")
                nc.scalar.activation(mt[:], lt[:], AF.Exp, scale=0.5)
                attn = pw.tile([128, 1024], F16, tag="wk",
                               name=f"attn_{h}_{I}")
                rs = psm.tile([128, 1], F32, tag="sm", name=f"rs_{h}_{I}")
                nc.scalar.activation(attn[:], mt[:], AF.Exp, accum_out=rs[:])
                return attn, rs

            def emit_norm(h, I, attn, rs):
                rc = psm.tile([128, 1], F32, tag="sm", name=f"rc_{h}_{I}")
                nc.vector.reciprocal(rc[:], rs[:])
                nc.gpsimd.tensor_scalar_mul(attn[:], attn[:], rc[:])

            def emit_transpose(h, I, attn):
                tps = psU.tile([128, 1024], F16, tag="pu",
                               name=f"tps_{h}_{I}")
                for J in range(NT):
                    js = slice(J * 128, (J + 1) * 128)
                    nc.tensor.transpose(tps[:, js], attn[:, js], id16[:])
                atT = pw.tile([128, 1024], F16, tag="wk", name=f"atT_{h}_{I}")
                nc.vector.tensor_copy(atT[:], tps[:])
                return atT

            def emit_av(h, I, atT):
                isl = slice(I * 128, (I + 1) * 128)
                avs = psU.tile([128, 128], F32, tag="pu",
                               name=f"avs_{h}_{I}")
                vsl = slice((h % 2) * 128, (h % 2) * 128 + 128)
                for J in range(NT):
                    js = slice(J * 128, (J + 1) * 128)
                    nc.tensor.matmul(avs[:], Vpp[h // 2][:, J, vsl],
                                     atT[:, js],
                                     start=(J == 0), stop=(J == NT - 1))
                prt = slice((h % 2) * 64, (h % 2) * 64 + 64)
                nc.vector.tensor_copy(OT_A[h // 2][prt, isl], avs[0:64, :])
                nc.vector.tensor_copy(OT_B[h // 2][prt, isl], avs[64:128, :])

            def emit_outproj(nh):
                ns = slice(nh * 512, (nh + 1) * 512)
                for part, bo_t, wo in (("r", bo_rt, wo_sn),
                                       ("i", bo_it, wo_st)):
                    if part == "r":
                        rhs = [OT_A[0], OT_A[1], OT_B[0], OT_B[1]]
                    else:
                        rhs = [OT_B[0], OT_B[1], OT_A[0], OT_A[1]]
                    for dt_ in range(4):
                        ds = slice(dt_ * 128, (dt_ + 1) * 128)
                        ops = psU.tile([128, 512], F32, tag="pu",
                                       name=f"ops_{part}_{dt_}_{nh}")
                        for j, rtt in enumerate(rhs):
                            nc.tensor.matmul(ops[:], wo[:, j, ds],
                                             rtt[:, ns],
                                             start=(j == 0), stop=(j == 3))
                        osb = pout.tile([128, 512], F32, tag="ot",
                                        name=f"osb_{part}_{dt_}_{nh}")
                        nc.scalar.add(osb[:], ops[:], bo_t[:, dt_:dt_ + 1])
                        dst = o_r if part == "r" else o_i
                        nc.sync.dma_start(
                            bass.AP(dst, dt_ * 128 * N + nh * 512,
                                    [[N, 128], [1, 512]]),
                            osb[:])

            # ---------------- pipelined main loop ----------------
            emit_proj("q", 0)
            emit_proj("k", 0)
            punits = [(k, h) for h in range(1, HPC) for k in ("q", "k")]
            vunits = [(p, J) for p in range(2) for J in range(NT)]

            flat = [(h, I) for h in range(HPC) for I in range(NT)]
            S_DOT, S_MAG, S_SOFT, S_NRM, S_TRA, S_AV = 3, 4, 5, 6, 7, 8
            qem, slotm, skwm, dotm, m2m, attm, atTm = ({} for _ in range(7))

            def at(s, d):
                k = s - d
                return flat[k] if 0 <= k < len(flat) else None

            # Per-step emission order sets each engine's queue order:
            #   PE:   dots+inject, transpose, AV, qrel mms, proj, vproj
            #   DVE:  mag2, norm, qe-r copies, proj-A/Vpp copies
            #   ACT:  soft (ln/exp/exp), qe-i copies, proj-K copies
            #   Pool: atT copy, OT copies, pair slot write
            #   sync: pair skew read
            for s in range(len(flat) + S_AV + 1):
                u = at(s, S_MAG)
                if u:
                    m2m[u] = emit_mag(*u, dotm.pop(u))
                u = at(s, S_SOFT)
                if u:
                    attm[u] = emit_soft(*u, m2m.pop(u))
                u = at(s, S_NRM)
                if u:
                    emit_norm(*u, *attm[u])
                u = at(s, S_TRA)
                if u:
                    atTm[u] = emit_transpose(*u, attm.pop(u)[0])
                u = at(s, S_AV)
                if u:
                    emit_av(*u, atTm.pop(u))
                    if u == (HPC - 1, 3):
                        emit_outproj(0)
                u = at(s, 0)
                if u:
                    if s % 2 == 0:
                        qem[s] = pqe.tile([128, 4608], F16, tag="qe",
                                          name=f"qe_{s}")
                    emit_qrel_qe(*u, qem[s - s % 2], s % 2)
                    if s % 2 == 1 or s == len(flat) - 1:
                        qe = qem.pop(s - s % 2)
                        slot = pdram.tile([128, 4608], F16, tag="qrev",
                                          name=f"qrev_{s}")
                        nc.gpsimd.dma_start(slot[:, :], qe[:])
                        slotm[s - s % 2] = slot
                u = at(s, S_DOT)
                if u:
                    k = s - S_DOT
                    dotm[u] = emit_dots(*u, skwm[flat[k - k % 2]], k % 2)
                    if k % 2:
                        skwm.pop(flat[k - 1])
                if s >= 2 and (s - 2) % 2 == 0 and (s - 2) in slotm:
                    skwm[flat[s - 2]] = emit_skwread(slotm.pop(s - 2))
                if punits:
                    emit_proj(*punits.pop(0))
                for _ in range(2):
                    if vunits:
                        emit_vproj(*vunits.pop(0))
            emit_outproj(1)

    nc.compile()
    return nc, mag2


def _prep_core_inputs(inputs, core):
    b, half = core // 2, core % 2
    x = inputs["x"]
    f16 = np.float16
    xt_r = np.ascontiguousarray(x[b, :, :, 0].T).astype(f16)
    xt_i = np.ascontiguousarray(x[b, :, :, 1].T).astype(f16)

    def pack_ab(wr, wi):
        a = np.empty((DIM, 512), np.float32)
        bb = np.empty((DIM, 512), np.float32)
        for hl in range(HPC):
            gh = half * HPC + hl
            cs = slice(gh * DH, (gh + 1) * DH)
            a[:, hl * 128:hl * 128 + 64] = wr[:, cs]
            a[:, hl * 128 + 64:hl * 128 + 128] = wi[:, cs]
            bb[:, hl * 128:hl * 128 + 64] = -wi[:, cs]
            bb[:, hl * 128 + 64:hl * 128 + 128] = wr[:, cs]
        return a.astype(f16), bb.astype(f16)

    wq_a, wq_b = pack_ab(inputs["wq_r"], inputs["wq_i"])
    wk_a, wk_b = pack_ab(inputs["wkv_r"][:, :512], inputs["wkv_i"][:, :512])
    wv_a, wv_b = pack_ab(inputs["wkv_r"][:, 512:], inputs["wkv_i"][:, 512:])

    rs = slice(half * 256, (half + 1) * 256)
    wo_st = np.concatenate(
        [inputs["wo_r"][rs, :], inputs["wo_i"][rs, :]], 0).astype(f16)
    wo_sn = np.concatenate(
        [inputs["wo_r"][rs, :], -inputs["wo_i"][rs, :]], 0).astype(f16)

    e = np.arange(2047)
    t_ext = inputs["rel_emb"][np.clip(e - 1023, -MAX_POS, MAX_POS) + MAX_POS]
    relrev = t_ext[::-1].astype(np.float32)    # [2047, 64]
    rel_r = np.zeros((128, 2048), f16)
    rel_i = np.zeros((128, 2048), f16)
    rel_r[0:64, 0:2047] = relrev.T
    rel_i[64:128, 0:2047] = -relrev.T

    bscale = 1.0 if half == 0 else 0.0
    bo_rt = np.ascontiguousarray(
        inputs["bo_r"].reshape(4, 128).T * bscale).astype(np.float32)
    bo_it = np.ascontiguousarray(
        inputs["bo_i"].reshape(4, 128).T * bscale).astype(np.float32)
    smask = np.concatenate(
        [np.full(64, SCALE, np.float32),
         np.full(64, -SCALE, np.float32)]).reshape(128, 1)

    return {
        "xt_r": xt_r, "xt_i": xt_i,
        "wq_a": wq_a, "wq_b": wq_b, "wk_a": wk_a, "wk_b": wk_b,
        "wv_a": wv_a, "wv_b": wv_b, "wo_st": wo_st, "wo_sn": wo_sn,
        "rel_r": rel_r, "rel_i": rel_i,
        "bo_rt": bo_rt, "bo_it": bo_it, "smask": smask,
    }


_last_results = {}


def kernel(**inputs):
    inputs = {k: np.asarray(v) for k, v in inputs.items()}
    nc, _ = build_module()
    in_maps = [_prep_core_inputs(inputs, c) for c in range(8)]
    res = run_bass_kernel_spmd(nc, in_maps, core_ids=list(range(8)))
    _last_results["res"] = res

    out = np.empty((B, N, DIM, 2), np.float32)
    for b in range(B):
        r = res.results[2 * b]["o_r"] + res.results[2 * b + 1]["o_r"]
        i = res.results[2 * b]["o_i"] + res.results[2 * b + 1]["o_i"]
        out[b, :, :, 0] = r.T
        out[b, :, :, 1] = i.T
    return out


# revision 30
# speedup vs baseline: 1.3217x; 1.0312x over previous
"""Complex-valued relative-position attention (nn_CAttention) on 8 TRN2 cores.

Sharding: batch (4) x head-half (2) -> 8 cores. Each core computes its
batch's projections for its 4 heads, full attention for those heads, and a
row-split partial output projection. Host sums the two partial outputs per
batch and restacks.

v3 design:
  - fp16 datapath end to end (x, projection weights, A/K stacks, rel
    tables): every matmul runs 1 cycle/row at any moving size.
  - Skew (rel-pos) terms are DMA-round-tripped through DRAM (diagonal AP
    read) then INJECTED into the dots PSUM group with identity matmuls, so
    the magnitude becomes two half-width custom DVE mag2 ops (dr^2+di^2)
    reading PSUM directly -- no adds on DVE.
  - qrel windows r+i share one SBUF qe tile per unit; PAIRS of units share
    one DRAM slot: one slot write + one diagonal read per two units.
  - Q/K projections for heads 1-3 are interleaved into the pipeline, so the
    startup is just x/weight loads + head-0 projections.
  - Output projection uses a negated-wo_i weight copy instead of negated AV
    stacks; osb bias-adds on ACT (idle at the tail).
  - Unified 8x1-bank PSUM pool for maximum slot fungibility.
"""
import functools
import numpy as np

import concourse.bass as bass
import concourse.bacc as bacc
import concourse.mybir as mybir
import concourse.tile as tile
from concourse.bass_utils import run_bass_kernel_spmd
from concourse.masks import make_identity

F32 = mybir.dt.float32
F16 = mybir.dt.float16
AF = mybir.ActivationFunctionType

HEADS, DH, MAX_POS = 8, 64, 512
B, N, DIM = 4, 1024, 512
HPC = 4            # heads per core
KT = 4             # dim k-tiles (512/128)
NT = 8             # n tiles (1024/128)
SCALE = DH ** (-0.5)


def register_custom_ops():
    from concourse import dve_ops
    from concourse.dve_spec import Spec, Src0, Src1, AluOp, Bin, lower, sq
    from concourse.dve_uop import DveOpSpec

    def reg(name, body, ref):
        for op in dve_ops.OPS:
            if op.name == name:
                return op
        spec = Spec(body=body, reference=ref)
        opcode = dve_ops._CUSTOM_DVE_ROW_BASE + len(dve_ops.OPS)
        shas = {}
        for ver in ("v3",):
            s = DveOpSpec(name=name, opcode=opcode,
                          uops=lower(spec, ver=ver), rd1_en=True)
            shas[ver] = s.sha(ver)
        op = dve_ops.DveOp(name, spec, subdim=False, uops_sha=shas)
        dve_ops._SUB_OPCODE_FOR_NAME[op.name] = opcode
        dve_ops.OPS.append(op)
        dve_ops.CUSTOM_DVE_SPECS[op.name] = op.spec
        return op

    mag2 = reg(
        "MAG2_ANT",
        Bin(AluOp.ADD, sq(Src0), sq(Src1)),
        lambda in0, in1, s0, s1, imm2: (
            in0.astype(np.float32) ** 2 + in1.astype(np.float32) ** 2),
    )
    return mag2


def c_lo(i_blk):
    return 896 - 128 * i_blk


@functools.cache
def build_module():
    import concourse.tile_utils as tile_utils
    if getattr(tile_utils, "max_sbuf_usage", 0) < 208 * 1024:
        tile_utils.max_sbuf_usage = 208 * 1024

    # Pin the ACT engine to the ln+exp table set: every activation this
    # kernel emits (Ln, Exp, Copy/Identity) lives in that one set, so the
    # greedy table-load pass emits exactly one LoadActFuncSet.
    import concourse.bacc as bacc_mod
    if not getattr(bacc_mod, "_ant_act_tables_pinned", False):
        orig_gat = bacc_mod.get_activation_tables

        def pinned_gat(arch):
            full = orig_gat(arch)
            out = {}
            for name, funcs in full.items():
                if name != "natural_log_exp_and_others":
                    funcs = funcs - {mybir.ActivationFunctionType.Ln,
                                     mybir.ActivationFunctionType.Exp}
                out[name] = funcs
            return out

        bacc_mod.get_activation_tables = pinned_gat
        bacc_mod._ant_act_tables_pinned = True

    mag2 = register_custom_ops()
    nc = bacc.Bacc("TRN2", target_bir_lowering=False, debug=False,
                   num_devices=8)

    din = {}
    for nm, shape, dt_ in [
        ("xt_r", [DIM, N], F16), ("xt_i", [DIM, N], F16),
        ("wq_a", [DIM, 512], F16), ("wq_b", [DIM, 512], F16),
        ("wk_a", [DIM, 512], F16), ("wk_b", [DIM, 512], F16),
        ("wv_a", [DIM, 512], F16), ("wv_b", [DIM, 512], F16),
        ("wo_st", [DIM, 512], F16), ("wo_sn", [DIM, 512], F16),
        ("rel_r", [128, 2048], F16), ("rel_i", [128, 2048], F16),
        ("bo_rt", [128, 4], F32), ("bo_it", [128, 4], F32),
        ("smask", [128, 1], F32),
    ]:
        din[nm] = nc.dram_tensor(nm, shape, dt_, kind="ExternalInput")
    o_r = nc.dram_tensor("o_r", [DIM, N], F32, kind="ExternalOutput")
    o_i = nc.dram_tensor("o_i", [DIM, N], F32, kind="ExternalOutput")

    with tile.TileContext(nc) as tc:
        with (
            tc.tile_pool(name="const", bufs=1) as cpool,
            tc.tile_pool(name="xw", bufs=8) as pxw,
            tc.tile_pool(name="ww", bufs=6) as pww,      # 1KB fp16 x/w
            tc.tile_pool(name="work", bufs=26) as pw,     # 2KB slots
            tc.tile_pool(name="qev", bufs=3) as pqe,      # 9KB pair slots
            tc.tile_pool(name="skw", bufs=3) as pskw,     # 8KB pair slots
            tc.tile_pool(name="stacks", bufs=12) as pstk,  # 2KB fp16 stacks
            tc.tile_pool(name="vpp", bufs=2) as pvp,      # 4KB
            tc.tile_pool(name="otp", bufs=4) as pot,      # 2KB fp16
            tc.tile_pool(name="outsb", bufs=4) as pout,
            tc.tile_pool(name="small", bufs=12) as psm,
            tc.tile_pool(name="psU", bufs=4, space="PSUM") as psU,
            tc.tile_pool(name="psD", bufs=4, space="PSUM") as psD,
            tc.tile_pool(name="dram", bufs=4, space="DRAM") as pdram,
        ):
            # ---------------- constants ----------------
            id16 = cpool.tile([128, 128], F16, tag="id16")
            make_identity(nc, id16[:])
            smask = cpool.tile([128, 1], F32, tag="smask")
            nc.sync.dma_start(smask[:], din["smask"][:, :])
            engs = (nc.sync, nc.scalar, nc.gpsimd)

            # x tiles: [128,1024] fp16 per (r/i, kt); weights [128,4,512]
            xt = {}
            qd = 0

            def load_w(nm):
                nonlocal qd
                t = pww.tile([128, 4, 512], F16, tag="w4", name=f"w_{nm}")
                engs[qd % 3].dma_start(
                    t[:, :, :],
                    bass.AP(din[nm], 0, [[512, 128], [512 * 128, 4], [1, 512]]))
                qd += 1
                return t

            def load_xt():
                nonlocal qd
                for nm in ("xt_r", "xt_i"):
                    for kt in range(KT):
                        t = pxw.tile([128, 1024], F16, tag="xw",
                                     name=f"{nm}_{kt}")
                        engs[qd % 3].dma_start(
                            t[:], bass.AP(din[nm], kt * 128 * N,
                                          [[N, 128], [1, 1024]]))
                        qd += 1
                        xt[(nm, kt)] = t

            wq = (load_w("wq_a"), load_w("wq_b"))
            load_xt()
            wk = (load_w("wk_a"), load_w("wk_b"))
            wv = (load_w("wv_a"), load_w("wv_b"))

            rel_r = cpool.tile([128, 2048], F16, tag="rel_r")
            rel_i = cpool.tile([128, 2048], F16, tag="rel_i")
            nc.sync.dma_start(rel_r[:], din["rel_r"][:, :])
            nc.scalar.dma_start(rel_i[:], din["rel_i"][:, :])
            bo_rt = cpool.tile([128, 4], F32, tag="bo_rt")
            bo_it = cpool.tile([128, 4], F32, tag="bo_it")
            nc.sync.dma_start(bo_rt[:], din["bo_rt"][:, :])
            nc.sync.dma_start(bo_it[:], din["bo_it"][:, :])
            wo_st = cpool.tile([128, 4, 512], F16, tag="wo_st")
            wo_sn = cpool.tile([128, 4, 512], F16, tag="wo_sn")
            nc.scalar.dma_start(
                wo_st[:, :, :],
                bass.AP(din["wo_st"], 0, [[512, 128], [512 * 128, 4], [1, 512]]))
            nc.scalar.dma_start(
                wo_sn[:, :, :],
                bass.AP(din["wo_sn"], 0, [[512, 128], [512 * 128, 4], [1, 512]]))

            # ---------------- projections (emitted per head) ----------------
            A16 = [None] * HPC
            Kn16 = [None] * HPC
            Ki16 = [None] * HPC

            def emit_proj(kind, h):
                wa, wb = wq if kind == "q" else wk
                hs = slice(h * 128, (h + 1) * 128)
                if kind == "q":
                    A16[h] = pstk.tile([128, 1024], F16, tag="stk",
                                       name=f"A{h}")
                else:
                    Kn16[h] = pstk.tile([128, 1024], F16, tag="stk",
                                        name=f"Kn{h}")
                    Ki16[h] = pstk.tile([128, 1024], F16, tag="stk",
                                        name=f"Ki{h}")
                for nh in range(2):
                    ns = slice(nh * 512, (nh + 1) * 512)
                    ps = psU.tile([128, 512], F32, tag="pu",
                                  name=f"ps{kind}_{h}_{nh}")
                    for kt in range(KT):
                        nc.tensor.matmul(ps[:], wa[:, kt, hs],
                                         xt[("xt_r", kt)][:, ns],
                                         start=(kt == 0), stop=False)
                    for kt in range(KT):
                        nc.tensor.matmul(ps[:], wb[:, kt, hs],
                                         xt[("xt_i", kt)][:, ns],
                                         start=False, stop=(kt == KT - 1))
                    if kind == "q":
                        nc.scalar.mul(A16[h][:, ns], ps[:], smask[:, 0:1])
                    else:
                        nc.scalar.copy(Kn16[h][:, ns], ps[:])
                        nc.scalar.copy(Ki16[h][0:64, ns], ps[64:128, :])
                        nc.scalar.mul(Ki16[h][64:128, ns],
                                      ps[0:64, :], -1.0)

            Vpp = [pvp.tile([128, 8, 256], F16, tag="vpp", name=f"Vpp{p}")
                   for p in range(2)]

            def emit_vproj(p, J):
                hs = slice(p * 256, (p + 1) * 256)
                xs = slice((J % 4) * 128 + (J // 4) * 512,
                           (J % 4) * 128 + (J // 4) * 512 + 128)
                vps = psU.tile([128, 256], F32, tag="pu",
                               name=f"vps_{p}_{J}")
                for kt in range(KT):
                    nc.tensor.matmul(vps[:], xt[("xt_r", kt)][:, xs],
                                     wv[0][:, kt, hs],
                                     start=(kt == 0), stop=False)
                for kt in range(KT):
                    nc.tensor.matmul(vps[:], xt[("xt_i", kt)][:, xs],
                                     wv[1][:, kt, hs],
                                     start=False, stop=(kt == KT - 1))
                nc.vector.tensor_copy(Vpp[p][:, J, :], vps[:])

            # OT stacks for the output projection (r and i AV halves)
            OT_A = [pot.tile([128, 1024], F16, tag="ot", name=f"OTA{t}")
                    for t in range(2)]
            OT_B = [pot.tile([128, 1024], F16, tag="ot", name=f"OTB{t}")
                    for t in range(2)]

            # ---------------- attention pipeline stages ----------------
            # Pairs of units share one qe tile / DRAM slot / skw tile:
            # layout per row: [u0-r 1152 | u0-i 1152 | u1-r 1152 | u1-i 1152]
            def emit_qrel_qe(h, I, qe, half):
                isl = slice(I * 128, (I + 1) * 128)
                lo = c_lo(I)
                for part, relt in ((0, rel_r), (1, rel_i)):
                    for c0, c1 in ((0, 512), (512, 1024), (1024, 1152)):
                        qps = psU.tile([128, c1 - c0], F32, tag="pu",
                                       name=f"qps{part}_{c0}_{h}_{I}")
                        nc.tensor.matmul(qps[:], A16[h][:, isl],
                                         relt[:, lo + c0:lo + c1],
                                         start=True, stop=True)
                        qs = slice(2304 * half + 1152 * part + c0,
                                   2304 * half + 1152 * part + c1)
                        if part == 0:
                            nc.vector.tensor_copy(qe[:, qs], qps[:])
                        else:
                            nc.scalar.copy(qe[:, qs], qps[:])

            def emit_skwread(slot):
                skw = pskw.tile([128, 4096], F16, tag="skw",
                                name=f"skw_{slot.name}")
                nc.sync.dma_start(
                    skw[:],
                    bass.AP(slot.tensor, 127,
                            [[4607, 128], [1152, 4], [1, 1024]]))
                return skw

            def emit_dots(h, I, skw, half):
                isl = slice(I * 128, (I + 1) * 128)
                out = []
                for part, K in ((0, Kn16[h]), (1, Ki16[h])):
                    for nh in range(2):
                        ns = slice(nh * 512, (nh + 1) * 512)
                        d = psD.tile([128, 512], F32, tag="pd",
                                     name=f"dps{part}_{nh}_{h}_{I}")
                        ss = 2048 * half + 1024 * part + 512 * nh
                        if part == 0:
                            # r-part: skew injected into PSUM by identity mm
                            nc.tensor.matmul(d[:], A16[h][:, isl], K[:, ns],
                                             start=True, stop=False)
                            nc.tensor.matmul(d[:], id16[:],
                                             skw[:, ss:ss + 512],
                                             start=False, stop=True)
                        else:
                            nc.tensor.matmul(d[:], A16[h][:, isl], K[:, ns],
                                             start=True, stop=True)
                        out.append(d)
                # i-part: skew add fused with the PSUM->SBUF staging (custom
                # DVE ops allow at most one PSUM operand, so MAG2 needs the
                # i-dots in SBUF anyway).
                ei = pw.tile([128, 1024], F16, tag="wk", name=f"ei_{h}_{I}")
                ss = 2048 * half + 1024
                nc.vector.tensor_add(ei[:, 0:512], out[2][:],
                                     skw[:, ss:ss + 512])
                nc.vector.tensor_add(ei[:, 512:1024], out[3][:],
                                     skw[:, ss + 512:ss + 1024])
                return out, ei

            def emit_mag(h, I, dps_ei):
                dps, ei = dps_ei
                m2 = pw.tile([128, 1024], F16, tag="wk", name=f"m2_{h}_{I}")
                for nh in range(2):
                    ns = slice(nh * 512, (nh + 1) * 512)
                    nc.vector._custom_dve(mag2, out=m2[:, ns],
                                          in0=ei[:, ns], in1=dps[nh][:])
                return m2

            def emit_soft(h, I, m2):
                # sqrt via exp(0.5*ln): single ACT table set for whole kernel
                lt = pw.tile([128, 1024], F16, tag="wk", name=f"lt_{h}_{I}")
                nc.scalar.activation(lt[:], m2[:], AF.Ln)
                mt = pw.tile([128, 1024], F16, tag="wk", name=f"mt_{h}_{I}")
                nc.scalar.activation(mt[:], lt[:], AF.Exp, scale=0.5)
                attn = pw.tile([128, 1024], F16, tag="wk",
                               name=f"attn_{h}_{I}")
                rs = psm.tile([128, 1], F32, tag="sm", name=f"rs_{h}_{I}")
                nc.scalar.activation(attn[:], mt[:], AF.Exp, accum_out=rs[:])
                return attn, rs

            def emit_norm(h, I, attn, rs):
                rc = psm.tile([128, 1], F32, tag="sm", name=f"rc_{h}_{I}")
                nc.vector.reciprocal(rc[:], rs[:])
                nc.gpsimd.tensor_scalar_mul(attn[:], attn[:], rc[:])

            def emit_transpose(h, I, attn):
                tps = psU.tile([128, 1024], F16, tag="pu",
                               name=f"tps_{h}_{I}")
                for J in range(NT):
                    js = slice(J * 128, (J + 1) * 128)
                    nc.tensor.transpose(tps[:, js], attn[:, js], id16[:])
                atT = pw.tile([128, 1024], F16, tag="wk", name=f"atT_{h}_{I}")
                nc.vector.tensor_copy(atT[:], tps[:])
                return atT

            def emit_av(h, I, atT):
                isl = slice(I * 128, (I + 1) * 128)
                avs = psU.tile([128, 128], F32, tag="pu",
                               name=f"avs_{h}_{I}")
                vsl = slice((h % 2) * 128, (h % 2) * 128 + 128)
                for J in range(NT):
                    js = slice(J * 128, (J + 1) * 128)
                    nc.tensor.matmul(avs[:], Vpp[h // 2][:, J, vsl],
                                     atT[:, js],
                                     start=(J == 0), stop=(J == NT - 1))
                prt = slice((h % 2) * 64, (h % 2) * 64 + 64)
                nc.vector.tensor_copy(OT_A[h // 2][prt, isl], avs[0:64, :])
                nc.vector.tensor_copy(OT_B[h // 2][prt, isl], avs[64:128, :])

            def emit_outproj(nh):
                ns = slice(nh * 512, (nh + 1) * 512)
                for part, bo_t, wo in (("r", bo_rt, wo_sn),
                                       ("i", bo_it, wo_st)):
                    if part == "r":
                        rhs = [OT_A[0], OT_A[1], OT_B[0], OT_B[1]]
                    else:
                        rhs = [OT_B[0], OT_B[1], OT_A[0], OT_A[1]]
                    for dt_ in range(4):
                        ds = slice(dt_ * 128, (dt_ + 1) * 128)
                        ops = psU.tile([128, 512], F32, tag="pu",
                                       name=f"ops_{part}_{dt_}_{nh}")
                        for j, rtt in enumerate(rhs):
                            nc.tensor.matmul(ops[:], wo[:, j, ds],
                                             rtt[:, ns],
                                             start=(j == 0), stop=(j == 3))
                        osb = pout.tile([128, 512], F32, tag="ot",
                                        name=f"osb_{part}_{dt_}_{nh}")
                        nc.scalar.add(osb[:], ops[:], bo_t[:, dt_:dt_ + 1])
                        dst = o_r if part == "r" else o_i
                        nc.sync.dma_start(
                            bass.AP(dst, dt_ * 128 * N + nh * 512,
                                    [[N, 128], [1, 512]]),
                            osb[:])

            # ---------------- pipelined main loop ----------------
            emit_proj("q", 0)
            emit_proj("k", 0)
            punits = [(k, h) for h in range(1, HPC) for k in ("q", "k")]
            vunits = [(p, J) for p in range(2) for J in range(NT)]

            flat = [(h, I) for h in range(HPC) for I in range(NT)]
            S_DOT, S_MAG, S_SOFT, S_NRM, S_TRA, S_AV = 4, 5, 6, 7, 8, 9
            qem, slotm, skwm, dotm, m2m, attm, atTm = ({} for _ in range(7))

            def at(s, d):
                k = s - d
                return flat[k] if 0 <= k < len(flat) else None

            # Per-step emission order sets each engine's queue order:
            #   PE:   dots+inject, transpose, AV, qrel mms, proj, vproj
            #   DVE:  mag2, norm, qe-r copies, proj-A/Vpp copies
            #   ACT:  soft (ln/exp/exp), qe-i copies, proj-K copies
            #   Pool: atT copy, OT copies, pair slot write
            #   sync: pair skew read
            for s in range(len(flat) + S_AV + 1):
                u = at(s, S_MAG)
                if u:
                    m2m[u] = emit_mag(*u, dotm.pop(u))
                u = at(s, S_SOFT)
                if u:
                    attm[u] = emit_soft(*u, m2m.pop(u))
                u = at(s, S_NRM)
                if u:
                    emit_norm(*u, *attm[u])
                u = at(s, S_TRA)
                if u:
                    atTm[u] = emit_transpose(*u, attm.pop(u)[0])
                u = at(s, S_AV)
                if u:
                    emit_av(*u, atTm.pop(u))
                    if u == (HPC - 1, 3):
                        emit_outproj(0)
                u = at(s, 0)
                if u:
                    if s % 2 == 0:
                        qem[s] = pqe.tile([128, 4608], F16, tag="qe",
                                          name=f"qe_{s}")
                    emit_qrel_qe(*u, qem[s - s % 2], s % 2)
                    if s % 2 == 1 or s == len(flat) - 1:
                        qe = qem.pop(s - s % 2)
                        slot = pdram.tile([128, 4608], F16, tag="qrev",
                                          name=f"qrev_{s}")
                        nc.gpsimd.dma_start(slot[:, :], qe[:])
                        slotm[s - s % 2] = slot
                u = at(s, S_DOT)
                if u:
                    k = s - S_DOT
                    dotm[u] = emit_dots(*u, skwm[flat[k - k % 2]], k % 2)
                    if k % 2:
                        skwm.pop(flat[k - 1])
                if s >= 2 and (s - 2) % 2 == 0 and (s - 2) in slotm:
                    skwm[flat[s - 2]] = emit_skwread(slotm.pop(s - 2))
                if punits:
                    emit_proj(*punits.pop(0))
                for _ in range(2):
                    if vunits:
                        emit_vproj(*vunits.pop(0))
            emit_outproj(1)

    nc.compile()
    return nc, mag2


def _prep_core_inputs(inputs, core):
    b, half = core // 2, core % 2
    x = inputs["x"]
    f16 = np.float16
    xt_r = np.ascontiguousarray(x[b, :, :, 0].T).astype(f16)
    xt_i = np.ascontiguousarray(x[b, :, :, 1].T).astype(f16)

    def pack_ab(wr, wi):
        a = np.empty((DIM, 512), np.float32)
        bb = np.empty((DIM, 512), np.float32)
        for hl in range(HPC):
            gh = half * HPC + hl
            cs = slice(gh * DH, (gh + 1) * DH)
            a[:, hl * 128:hl * 128 + 64] = wr[:, cs]
            a[:, hl * 128 + 64:hl * 128 + 128] = wi[:, cs]
            bb[:, hl * 128:hl * 128 + 64] = -wi[:, cs]
            bb[:, hl * 128 + 64:hl * 128 + 128] = wr[:, cs]
        return a.astype(f16), bb.astype(f16)

    wq_a, wq_b = pack_ab(inputs["wq_r"], inputs["wq_i"])
    wk_a, wk_b = pack_ab(inputs["wkv_r"][:, :512], inputs["wkv_i"][:, :512])
    wv_a, wv_b = pack_ab(inputs["wkv_r"][:, 512:], inputs["wkv_i"][:, 512:])

    rs = slice(half * 256, (half + 1) * 256)
    wo_st = np.concatenate(
        [inputs["wo_r"][rs, :], inputs["wo_i"][rs, :]], 0).astype(f16)
    wo_sn = np.concatenate(
        [inputs["wo_r"][rs, :], -inputs["wo_i"][rs, :]], 0).astype(f16)

    e = np.arange(2047)
    t_ext = inputs["rel_emb"][np.clip(e - 1023, -MAX_POS, MAX_POS) + MAX_POS]
    relrev = t_ext[::-1].astype(np.float32)    # [2047, 64]
    rel_r = np.zeros((128, 2048), f16)
    rel_i = np.zeros((128, 2048), f16)
    rel_r[0:64, 0:2047] = relrev.T
    rel_i[64:128, 0:2047] = -relrev.T

    bscale = 1.0 if half == 0 else 0.0
    bo_rt = np.ascontiguousarray(
        inputs["bo_r"].reshape(4, 128).T * bscale).astype(np.float32)
    bo_it = np.ascontiguousarray(
        inputs["bo_i"].reshape(4, 128).T * bscale).astype(np.float32)
    smask = np.concatenate(
        [np.full(64, SCALE, np.float32),
         np.full(64, -SCALE, np.float32)]).reshape(128, 1)

    return {
        "xt_r": xt_r, "xt_i": xt_i,
        "wq_a": wq_a, "wq_b": wq_b, "wk_a": wk_a, "wk_b": wk_b,
        "wv_a": wv_a, "wv_b": wv_b, "wo_st": wo_st, "wo_sn": wo_sn,
        "rel_r": rel_r, "rel_i": rel_i,
        "bo_rt": bo_rt, "bo_it": bo_it, "smask": smask,
    }


_last_results = {}


def kernel(**inputs):
    inputs = {k: np.asarray(v) for k, v in inputs.items()}
    nc, _ = build_module()
    in_maps = [_prep_core_inputs(inputs, c) for c in range(8)]
    res = run_bass_kernel_spmd(nc, in_maps, core_ids=list(range(8)))
    _last_results["res"] = res

    out = np.empty((B, N, DIM, 2), np.float32)
    for b in range(B):
        r = res.results[2 * b]["o_r"] + res.results[2 * b + 1]["o_r"]
        i = res.results[2 * b]["o_i"] + res.results[2 * b + 1]["o_i"]
        out[b, :, :, 0] = r.T
        out[b, :, :, 1] = i.T
    return out
